# revision 8
# baseline (speedup 1.0000x reference)
"""AucLoss on 8 TRN2 NeuronCores (Bass SPMD kernel).

Reference (B=8192, C=2048, GAMA=0.3, UNK=0):
    s = sigmoid(x);  pos_i = s[i, y_i];  valid_i = (y_i != 0)
    neg_j = max_c s[j, c] over c not in {y_j, 0};  t_j = neg_j + GAMA
    sq_sum = sum_{i valid, j} [t_j > pos_i] * (t_j - pos_i)^2
    loss = sq_sum / (p_count + 1) / (B + 1)

Data-parallel over the batch: each core streams a 1024x2048 shard of x
once (8 MB, the DMA roofline at ~360 GB/s governs the runtime; phase 1
is DMA-bound at 2913ns per 128-row block). Per block, in logit space
(sigmoid is monotone so the masked row-max commutes with it):
  DVE : xz[0:872)  = (iota != y) * x with sum-accum        (STT, 1x)
        row-max of xz[1:872) and of xm[872:2048)           (TS, 2x)
        label mask for [872:2048) as bf16                  (TS, 4x)
  Pool: xm[872:2048) = x + mask  (the Pool engine has no
        scalar_tensor_tensor / accum opcodes, so it takes the
        baseline-style masked-add share)
  ACT : rowsum x[0:872) + relu(-xm-8000) sum over [872:2048)
  pos_logit = (rowsum - sum(xz)) + 192*[y >= 872] - relu_sum
    (x[y] recovered exactly: zeroed-column sum difference in the DVE
    region, the -8192-offset relu trick in the Pool region)
The pairwise term needs no O(B^2) work:
    sum_{i valid, j} (t_j - pos_i)^2 = B*S2 - 2*T1*S1 + T2*P
with per-partition stats packed in one [128,6,8] tile, reduced by two
strided tensor_reduce ops, cross-partition via one PE ones-matmul
(sums) + transpose/reduce/transpose (maxes), and shipped through a
SINGLE 32-byte-per-core AllGather (the baseline shipped 2x6KB). The
margin indicator is enforced by a runtime guard (max v*pos >= min t,
never true in this regime); the full [t|vpos] vectors are gathered
unconditionally (a collective inside tc.If deadlocks the runtime) but
only read inside the guarded O(B^2) correction branch.

Measured (TimelineSim, collectives modeled as local copies, same
methodology as the 55782ns baseline): 44861 ns, rel err 0.0.
"""

from contextlib import ExitStack

import numpy as np

import concourse.bass as bass
import concourse.mybir as mybir
import concourse.tile as tile
from concourse.vector_clock import ScopedClock

F32 = mybir.dt.float32
I32 = mybir.dt.int32
ALU = mybir.AluOpType
ACTF = mybir.ActivationFunctionType

B_FULL, C_FULL, N_CORES, GAMA = 8192, 2048, 8, 0.3


class _PatchedTileContext(tile.TileContext):
    """TileContext whose tail drain splits sem waits one per instruction."""

    def _drain_and_barrier(self, tick_clock, wait_clock):
        nc = self.nc
        drain_inst = nc.sync.drain()
        wait_clock.add_sem_waits(
            drain_inst.ins, ScopedClock({None: tick_clock.global_clock})
        )
        si = drain_inst.ins.sync_info
        if si is not None and si.on_wait and len(si.on_wait) > 1:
            extra = list(si.on_wait[1:])
            del si.on_wait[1:]
            for w in extra:
                ni = nc.sync.nop()
                nsi = ni.ins.sync_info
                if nsi is None:
                    ni.ins.sync_info = mybir.SyncInfo(on_wait=[w], on_update=[])
                else:
                    nsi.on_wait.append(w)

        nc.all_engine_barrier()
        assert self.sems is not None
        popped = nc._tile_sem_poison_stack.pop()
        assert popped is self._sem_poison
        nc.clear_and_free_semaphores(list(self.sems.allocated().values()))
        nc.all_engine_barrier()


def _split_multi_waits(nc):
    n = 0
    for f in nc.m.functions:
        for bb in f.blocks:
            out = []
            for ins in bb.instructions:
                si = ins.sync_info
                if si is not None and si.on_wait and len(si.on_wait) > 1:
                    extra = list(si.on_wait[:-1])
                    del si.on_wait[:-1]
                    for w in extra:
                        n += 1
                        out.append(mybir.InstNoOp(
                            name=f"waitnop_{n}",
                            engine=ins.engine,
                            ins=[],
                            outs=[],
                            sync_info=mybir.SyncInfo(on_wait=[w], on_update=[]),
                        ))
                out.append(ins)
            bb.instructions[:] = out
    return n


def _build(B=B_FULL, C=C_FULL, n_cores=N_CORES, gama=GAMA,
           fake_collective=False, split_waits=True, skip_correction=False,
           iota_psum=False):
    R = B // n_cores
    nb = R // 128
    assert R % 128 == 0
    CS = 872           # D region [0:CS): DVE STT-zeroing; P region
    B7S = 1792         # [CS:C): DVE bf16 mask + Pool add (TensorScalar
    MASKVAL = -8192.0  # accum and STT are illegal on Pool)

    nc = bass.Bass("TRN2", target_bir_lowering=False, debug=False,
                   num_devices=n_cores)
    x_ap = nc.dram_tensor("x", [R, C], F32, kind="ExternalInput").ap()
    y_ap = nc.dram_tensor("yt", [128, nb], I32, kind="ExternalInput").ap()
    if iota_psum:
        iota_ap = nc.dram_tensor("iota1", [1, C], F32,
                                 kind="ExternalInput").ap()
    else:
        iota_ap = nc.dram_tensor("iota2", [128, C],
                                 mybir.dt.int16, kind="ExternalInput").ap()
    ident_ap = nc.dram_tensor("ident", [128, 128], F32,
                              kind="ExternalInput").ap()
    out_ap = nc.dram_tensor("out", [1], F32, kind="ExternalOutput").ap()

    groups = [list(range(n_cores))]

    with _PatchedTileContext(nc) as tc:
        with ExitStack() as stk:
            persist = stk.enter_context(tc.tile_pool(name="persist", bufs=1))
            dram = stk.enter_context(
                tc.tile_pool(name="dram", bufs=1, space="DRAM"))
            psum = stk.enter_context(
                tc.tile_pool(name="psum", bufs=1, space="PSUM"))

            y32 = persist.tile([128, nb], I32)
            nc.sync.dma_start(out=y32[:], in_=y_ap)

            # host-supplied constants (custom gpsimd ucode such as
            # InstIota does not codegen in this walrus build). With
            # iota_psum the 512KB iota DMA becomes an 8KB row broadcast
            # through a PE ones-matmul into PSUM.
            ones = persist.tile([128, 1], F32)
            nc.vector.memset(ones[:], 1.0)
            if iota_psum:
                iota1 = persist.tile([1, C], F32)
                nc.sync.dma_start(out=iota1[:], in_=iota_ap)
                ones1 = persist.tile([1, 128], F32)
                nc.vector.memset(ones1[:], 1.0)
                iota2 = psum.tile([128, C], F32, tag="iotaP")
                nc.tensor.matmul(iota2[:], ones1[:], iota1[:], start=True,
                                 stop=True)
            else:
                iota2 = persist.tile([128, C], mybir.dt.int16)
                nc.sync.dma_start(out=iota2[:], in_=iota_ap)
            ident = persist.tile([128, 128], F32)
            nc.sync.dma_start(out=ident[:], in_=ident_ap)

            valid = persist.tile([128, nb], F32)
            nc.vector.tensor_scalar(valid[:], y32[:], 0, None, ALU.not_equal)
            yf = persist.tile([128, nb], F32)
            nc.vector.tensor_copy(yf[:], y32[:])

            # phase-1 accumulators
            negl = persist.tile([128, nb], F32)
            neglD = persist.tile([128, nb], F32)
            neglP = persist.tile([128, nb], F32)
            sxzD = persist.tile([128, nb], F32)
            sxD = persist.tile([128, nb], F32)
            rlu = persist.tile([128, nb], F32)
            fillacc = persist.tile([128, 8], F32)
            bn8000 = persist.tile([128, 1], F32)
            nc.vector.memset(bn8000[:], -8000.0)
            inP = persist.tile([128, nb], F32)
            nc.vector.tensor_scalar(inP[:], yf[:], float(CS), None,
                                    ALU.is_ge)

            # per-partition stats; 0 sum t, 1 sum t^2, 2 sum v*pos,
            # 3 sum v*pos^2, 4 p_count, 5 max v*pos, 6 max -t, 7 pad
            ls8 = persist.tile([128, 8], F32)
            jkv = persist.tile([128, nb], F32)
            nc.vector.tensor_scalar(jkv[:], valid[:], 0.0, None, ALU.add,
                                    ALU.add, accum_out=ls8[:, 4:5])
            nc.vector.memset(ls8[:, 7:8], 0.0)

            # warm the sigmoid table early (real hw: ~2.7us PSEUDO_LOAD)
            warm = persist.tile([1, 1], F32)
            nc.scalar.activation(warm[:], ones[0:1, 0:1], ACTF.Sigmoid)

            # shared junk outputs (single-engine writers, in-order queues)
            jmax = persist.tile([128, C - 1], F32)
            jact = persist.tile([128, C], F32)
            jrlu = persist.tile([128, C - CS], F32)

            # packed per-block stat vectors: rows [t, vp, t2, vp2,
            # vp-copy, -t]; rows 0:2 double as the If-gather payload
            stat_v = persist.tile([128, 6, nb], F32)
            t_ = stat_v[:, 0, :]
            vpos = stat_v[:, 1, :]
            t2_ = stat_v[:, 2, :]
            vp2_ = stat_v[:, 3, :]
            vpc_ = stat_v[:, 4, :]
            nt_ = stat_v[:, 5, :]
            lstats = stat_v[:, 0:2, :]
            negs = persist.tile([128, nb], F32)
            ptmp = persist.tile([128, nb], F32)
            posm = persist.tile([128, nb], F32)
            pos = persist.tile([128, nb], F32)
            pwarm = psum.tile([1, 1], F32, tag="pwarm")

            def emit_stats():
                nc.vector.tensor_tensor(out=negl[:], in0=neglD[:],
                                        in1=neglP[:], op=ALU.max)
                nc.scalar.activation(negs[:], negl[:], ACTF.Sigmoid)
                # pos_logit = (sxD - sxzD) + 192*inP - rlu
                nc.vector.scalar_tensor_tensor(
                    ptmp[:], sxzD[:], -1.0, sxD[:], ALU.mult, ALU.add)
                nc.vector.scalar_tensor_tensor(
                    posm[:], inP[:], 192.0, ptmp[:], ALU.mult, ALU.add)
                nc.vector.tensor_tensor(out=posm[:], in0=posm[:],
                                        in1=rlu[:], op=ALU.subtract)
                nc.scalar.activation(pos[:], posm[:], ACTF.Sigmoid)
                nc.gpsimd.tensor_scalar(t_, negs[:], float(gama), None,
                                        ALU.add)
                nc.gpsimd.tensor_tensor(out=t2_, in0=t_, in1=t_,
                                        op=ALU.mult)
                nc.gpsimd.tensor_scalar(nt_, t_, -1.0, None, ALU.mult)
                nc.vector.tensor_tensor(out=vpos, in0=pos[:], in1=valid[:],
                                        op=ALU.mult)
                nc.vector.tensor_tensor(out=vp2_, in0=vpos, in1=vpos,
                                        op=ALU.mult)
                nc.vector.tensor_copy(vpc_, vpos)

            # ---- phase 1 ----            # ---- phase 1 ----
            xp = stk.enter_context(tc.tile_pool(name="xp", bufs=3))
            zp = stk.enter_context(tc.tile_pool(name="zp", bufs=3))
            if True:
                for b in range(nb):
                    sub = 2 if b == nb - 1 else 1
                    xb = xp.tile([128, C], F32, tag="x")
                    xz = zp.tile([128, CS], F32, tag="xz")
                    mask = zp.tile([128, C - CS], mybir.dt.bfloat16,
                                   tag="mask")
                    xm = zp.tile([128, C - CS], F32, tag="xm")
                    if sub == 1:
                        nc.sync.dma_start(
                            out=xb[:], in_=x_ap[128 * b:128 * (b + 1), :])
                        # P region: bf16 mask (DVE 4x) + add on Pool
                        nc.vector.tensor_scalar(
                            mask[:], iota2[:, CS:C], yf[:, b:b + 1],
                            MASKVAL, ALU.is_equal, ALU.mult)
                        nc.gpsimd.tensor_tensor(
                            out=xm[:], in0=xb[:, CS:C], in1=mask[:],
                            op=ALU.add)
                        # D region: STT zeroing with sum accum
                        nc.vector.scalar_tensor_tensor(
                            xz[:], iota2[:, 0:CS], yf[:, b:b + 1],
                            xb[:, 0:CS], ALU.not_equal, ALU.mult,
                            accum_out=sxzD[:, b:b + 1])
                        nc.vector.tensor_scalar(
                            jmax[:, 0:CS - 1], xz[:, 1:CS], 0.0, None,
                            ALU.add, ALU.max, accum_out=neglD[:, b:b + 1])
                        nc.vector.tensor_scalar(
                            jmax[:, CS - 1:C - 1], xm[:], 0.0, None,
                            ALU.add, ALU.max, accum_out=neglP[:, b:b + 1])
                        nc.scalar.activation(
                            jact[:, 0:CS], xb[:, 0:CS], ACTF.Copy,
                            accum_out=sxD[:, b:b + 1])
                        nc.scalar.activation(
                            jrlu[:], xm[:], ACTF.Relu, bias=bn8000[:],
                            scale=-1.0, accum_out=rlu[:, b:b + 1])
                        nc.tensor.matmul(pwarm[:], ones[0:1, 0:1],
                                         ones[0:1, 0:1], start=True,
                                         stop=True)
                    else:
                        # block 7 split [0:B7S) + [B7S:C) to shrink the tail
                        nc.sync.dma_start(
                            out=xb[:, 0:B7S],
                            in_=x_ap[128 * b:128 * (b + 1), 0:B7S])
                        nc.sync.dma_start(
                            out=xb[:, B7S:C],
                            in_=x_ap[128 * b:128 * (b + 1), B7S:C])
                        # h1: D region + P part [CS:B7S)
                        nc.vector.tensor_scalar(
                            mask[:, 0:B7S - CS], iota2[:, CS:B7S],
                            yf[:, b:b + 1], MASKVAL, ALU.is_equal, ALU.mult)
                        nc.gpsimd.tensor_tensor(
                            out=xm[:, 0:B7S - CS], in0=xb[:, CS:B7S],
                            in1=mask[:, 0:B7S - CS], op=ALU.add)
                        nc.vector.scalar_tensor_tensor(
                            xz[:], iota2[:, 0:CS], yf[:, b:b + 1],
                            xb[:, 0:CS], ALU.not_equal, ALU.mult,
                            accum_out=sxzD[:, b:b + 1])
                        nc.vector.tensor_scalar(
                            jmax[:, 0:CS - 1], xz[:, 1:CS], 0.0, None,
                            ALU.add, ALU.max, accum_out=neglD[:, b:b + 1])
                        nc.vector.tensor_scalar(
                            jmax[:, CS - 1:B7S - 1], xm[:, 0:B7S - CS],
                            0.0, None, ALU.add, ALU.max,
                            accum_out=fillacc[:, 0:1])
                        nc.scalar.activation(
                            jact[:, 0:CS], xb[:, 0:CS], ACTF.Copy,
                            accum_out=sxD[:, b:b + 1])
                        nc.scalar.activation(
                            jrlu[:, 0:B7S - CS], xm[:, 0:B7S - CS],
                            ACTF.Relu, bias=bn8000[:], scale=-1.0,
                            accum_out=fillacc[:, 2:3])
                        # h2: all P
                        nc.vector.tensor_scalar(
                            mask[:, B7S - CS:], iota2[:, B7S:C],
                            yf[:, b:b + 1], MASKVAL, ALU.is_equal, ALU.mult)
                        nc.gpsimd.tensor_tensor(
                            out=xm[:, B7S - CS:], in0=xb[:, B7S:C],
                            in1=mask[:, B7S - CS:], op=ALU.add)
                        nc.vector.tensor_scalar(
                            jmax[:, B7S - 1:C - 1], xm[:, B7S - CS:],
                            0.0, None, ALU.add, ALU.max,
                            accum_out=fillacc[:, 1:2])
                        nc.scalar.activation(
                            jrlu[:, B7S - CS:], xm[:, B7S - CS:],
                            ACTF.Relu, bias=bn8000[:], scale=-1.0,
                            accum_out=fillacc[:, 3:4])
                        # combines
                        nc.vector.tensor_tensor(
                            out=neglP[:, b:b + 1], in0=fillacc[:, 0:1],
                            in1=fillacc[:, 1:2], op=ALU.max)
                        nc.gpsimd.tensor_tensor(
                            out=rlu[:, b:b + 1], in0=fillacc[:, 2:3],
                            in1=fillacc[:, 3:4], op=ALU.add)
                emit_stats()

            # ---- phase 2: two packed reduces into ls8 ----
            nc.vector.tensor_reduce(
                ls8[:, 0:4].rearrange("p s -> p s ()"), stat_v[:, 0:4, :],
                mybir.AxisListType.X, ALU.add)
            nc.vector.tensor_reduce(
                ls8[:, 5:7].rearrange("p s -> p s ()"), stat_v[:, 4:6, :],
                mybir.AxisListType.X, ALU.max)

            # ---- cross-partition: matmul for sums, transpose+reduce+
            # transpose for maxes (all partition-offset-0 accesses; the
            # walrus verifier rejects offsets not on a quadrant boundary)
            pstats = psum.tile([1, 5], F32, tag="pstats")
            nc.tensor.matmul(pstats[:], ones[:], ls8[:, 0:5], start=True,
                             stop=True)
            pmt = psum.tile([2, 128], F32, tag="pmt")
            nc.tensor.transpose(pmt[:], ls8[:, 5:7], ident[:])
            gm = persist.tile([2, 1], F32)
            nc.vector.tensor_reduce(gm[:], pmt[:], mybir.AxisListType.X,
                                    ALU.max)
            pgt = psum.tile([1, 2], F32, tag="pgt")
            nc.tensor.transpose(pgt[:], gm[:], ident[0:2, 0:2])
            chunk = persist.tile([1, 8], F32)
            nc.vector.tensor_copy(chunk[0:1, 0:5], pstats[:])
            nc.vector.tensor_copy(chunk[0:1, 5:7], pgt[:])
            nc.vector.memset(chunk[0:1, 7:8], 0.0)

            chunkd = dram.tile([8], F32, name="chunkd")
            nc.sync.dma_start(out=chunkd[:].rearrange("(p s) -> p s", p=1),
                              in_=chunk[:])
            agS = dram.tile([n_cores * 8], F32, name="agS")
            if fake_collective:
                # model: one DMA materializing the gathered buffer
                nc.sync.dma_start(
                    out=agS[:].rearrange("(k s) -> k s", k=n_cores),
                    in_=bass.AP(chunkd[:].tensor, 0, [[0, n_cores], [1, 8]]))
            else:
                nc.gpsimd.collective_compute(
                    "AllGather", ALU.bypass, replica_groups=groups,
                    ins=[chunkd.opt()], outs=[agS.opt()])
            gstat = persist.tile([1, n_cores, 8], F32)
            nc.sync.dma_start(
                out=gstat[:],
                in_=agS[:].rearrange("(o k s) -> o k s", o=1, k=n_cores))

            # global sums / maxes across the 8 gathered slots (stat-major
            # views so one reduce covers all 8 stats)
            gadd = persist.tile([1, 8], F32)
            gmaxt = persist.tile([1, 8], F32)
            gview = gstat[:].rearrange("o k s -> o s k")
            nc.vector.tensor_reduce(gadd[:].rearrange("o s -> o s ()"),
                                    gview, mybir.AxisListType.X, ALU.add)
            nc.vector.tensor_reduce(gmaxt[:].rearrange("o s -> o s ()"),
                                    gview, mybir.AxisListType.X, ALU.max)
            g = gadd
            Pk = g[0:1, 4:5]

            # main = B*S2 - 2*T1*S1 + T2*P
            m2 = persist.tile([1, 1], F32)
            nc.vector.scalar_tensor_tensor(m2[:], g[0:1, 0:1], -2.0,
                                           g[0:1, 1:2], ALU.mult, ALU.mult)
            m3 = persist.tile([1, 1], F32)
            nc.vector.tensor_tensor(out=m3[:], in0=g[0:1, 2:3], in1=Pk,
                                    op=ALU.mult)
            m13 = persist.tile([1, 1], F32)
            nc.vector.scalar_tensor_tensor(m13[:], g[0:1, 3:4], float(B),
                                           m3[:], ALU.mult, ALU.add)
            main = persist.tile([1, 1], F32)
            nc.vector.tensor_tensor(out=main[:], in0=m13[:], in1=m2[:],
                                    op=ALU.add)

            corr = persist.tile([1, 1], F32)
            nc.vector.memset(corr[:], 0.0)
            den = persist.tile([1, 1], F32)
            nc.vector.tensor_scalar(den[:], Pk, 1.0, float(B) + 1.0,
                                    ALU.add, ALU.mult)
            rec = persist.tile([1, 1], F32)
            nc.vector.reciprocal(rec[:], den[:])

            if not skip_correction:
                # guard: max(v*pos) >= min(t)  <=>  g5 + g6 >= 0
                gsum = persist.tile([1, 1], F32)
                nc.vector.tensor_tensor(out=gsum[:], in0=gmaxt[0:1, 5:6],
                                        in1=gmaxt[0:1, 6:7], op=ALU.add)
                flag = persist.tile([1, 1], I32)
                nc.vector.tensor_scalar(flag[:], gsum[:], 0.0, None,
                                        ALU.is_ge)
                # the full [t | vpos] vectors are gathered unconditionally
                # (a collective inside the If deadlocks the runtime); the
                # readback + O(B^2) work stay behind the branch
                CHV = 2 * 128 * nb
                chunkv = dram.tile([CHV], F32, name="chunkv")
                nc.sync.dma_start(
                    out=chunkv[:].rearrange("(s p b) -> p s b",
                                            s=2, p=128),
                    in_=lstats[:])
                agV = dram.tile([n_cores * CHV], F32, name="agV")
                if fake_collective:
                    nc.sync.dma_start(
                        out=agV[0:CHV],
                        in_=chunkv[:])
                else:
                    nc.gpsimd.collective_compute(
                        "AllGather", ALU.bypass, replica_groups=groups,
                        ins=[chunkv.opt()], outs=[agV.opt()])
                tmp = nc.alloc_registers(f"corr_flag_{nc.next_id()}",
                                         mybir.ALL_ENGINES)
                nc.regs_load(tmp, flag[0:1, 0:1])
                rv = nc.snap(tmp, donate=True, min_val=0, max_val=1)
                with tc.If(rv == 1):
                    nbg = n_cores * nb
                    gall = persist.tile([128, 2, nbg], F32)
                    for s in range(2):
                        nc.sync.dma_start(
                            out=gall[:, s, :].rearrange(
                                "p (k b) -> p k b", k=n_cores),
                            in_=agV[:].rearrange(
                                "(k s p b) -> s p k b", k=n_cores, s=2,
                                p=128)[s])
                    tall = gall[:, 0, :]
                    vposall = gall[:, 1, :]
                    # broadcast all B t values to one [128, B] tile
                    tflat = dram.tile([B], F32)
                    nc.sync.dma_start(
                        out=tflat[:].rearrange("(p b) -> p b", p=128),
                        in_=tall)
                    tb1 = persist.tile([1, B], F32)
                    nc.sync.dma_start(out=tb1[:], in_=tflat[:].rearrange(
                        "(o n) -> o n", o=1))
                    tb = persist.tile([128, B], F32)
                    onesb = persist.tile([1, 128], F32)
                    nc.vector.memset(onesb[:], 1.0)
                    CBC = 512
                    for j in range(0, B, CBC):
                        pbc = psum.tile([128, CBC], F32, tag="pbc")
                        nc.tensor.matmul(pbc[:], onesb[:],
                                         tb1[0:1, j:j + CBC],
                                         start=True, stop=True)
                        nc.vector.tensor_copy(tb[:, j:j + CBC], pbc[:])
                    cacc = persist.tile([128, nbg], F32)
                    with tc.tile_pool(name="cp", bufs=1) as cp:
                        for c in range(nbg):
                            r1 = cp.tile([128, B], mybir.dt.bfloat16,
                                         tag="r1")
                            nc.scalar.activation(r1[:], tb[:], ACTF.Relu,
                                                 bias=vposall[:, c:c + 1],
                                                 scale=-1.0)
                            r2 = cp.tile([128, B], mybir.dt.bfloat16,
                                         tag="r2")
                            nc.scalar.activation(r2[:], r1[:], ACTF.Square,
                                                 accum_out=cacc[:, c:c + 1])
                    cp1 = persist.tile([128, 1], F32)
                    nc.vector.tensor_reduce(cp1[:], cacc[:],
                                            mybir.AxisListType.X, ALU.add)
                    pc = psum.tile([1, 1], F32, tag="pc")
                    nc.tensor.matmul(pc[:], ones[:], cp1[:], start=True,
                                     stop=True)
                    nc.vector.tensor_copy(corr[:], pc[0:1, 0:1])

            total = persist.tile([1, 1], F32)
            nc.vector.tensor_tensor(out=total[:], in0=main[:], in1=corr[:],
                                    op=ALU.subtract)
            loss = persist.tile([1, 1], F32)
            nc.vector.tensor_tensor(out=loss[:], in0=total[:], in1=rec[:],
                                    op=ALU.mult)
            nc.sync.dma_start(out=out_ap[0:1], in_=loss[0:1, 0:1])

    if split_waits:
        _split_multi_waits(nc)
    return nc


class _CachedSpmdExec:
    """Build once, execute many times via PJRT shard_map (axon path)."""

    def __init__(self, nc, n_cores):
        import jax
        from jax.sharding import Mesh, PartitionSpec
        from jax.experimental.shard_map import shard_map
        from concourse import bass2jax

        bass2jax.install_neuronx_cc_hook()
        self.n_cores = n_cores
        assert nc.dbg_addr is None

        partition_name = (nc.partition_id_tensor.name
                          if nc.partition_id_tensor else None)
        in_names, out_names, out_avals, zero_shapes = [], [], [], []
        for alloc in nc.m.functions[0].allocations:
            if not isinstance(alloc, mybir.MemoryLocationSet):
                continue
            name = alloc.memorylocations[0].name
            if alloc.kind == "ExternalInput":
                if name != partition_name:
                    in_names.append(name)
            elif alloc.kind == "ExternalOutput":
                out_names.append(name)
                shape = tuple(alloc.tensor_shape)
                dtype = mybir.dt.np(alloc.dtype)
                out_avals.append(jax.core.ShapedArray(shape, dtype))
                zero_shapes.append((shape, dtype))
        self.n_params = len(in_names)
        self.in_names = list(in_names)
        self.out_names = out_names
        self.zero_shapes = zero_shapes
        all_in_names = in_names + out_names
        if partition_name is not None:
            all_in_names.append(partition_name)

        n_outs = len(out_names)
        donate = tuple(range(self.n_params, self.n_params + n_outs))

        def _body(*args):
            operands = list(args)
            if partition_name is not None:
                operands.append(bass2jax.partition_id_tensor())
            outs = bass2jax._bass_exec_p.bind(
                *operands,
                out_avals=tuple(out_avals),
                in_names=tuple(all_in_names),
                out_names=tuple(out_names),
                lowering_input_output_aliases=(),
                sim_require_finite=True,
                sim_require_nnan=True,
                nc=nc,
            )
            return tuple(outs)

        devices = jax.devices()[:n_cores]
        assert len(devices) == n_cores
        mesh = Mesh(np.asarray(devices), ("core",))
        in_specs = (PartitionSpec("core"),) * (self.n_params + n_outs)
        out_specs = (PartitionSpec("core"),) * n_outs
        self.sharded = jax.jit(
            shard_map(_body, mesh=mesh, in_specs=in_specs,
                      out_specs=out_specs, check_rep=False),
            donate_argnums=donate, keep_unused=True,
        )

    def __call__(self, in_maps):
        n = self.n_cores
        concat_in = [
            np.concatenate([np.asarray(in_maps[c][name]) for c in range(n)],
                           axis=0)
            for name in self.in_names
        ]
        concat_zeros = [
            np.zeros((n * s[0], *s[1:]), d) for (s, d) in self.zero_shapes
        ]
        out_arrs = [np.asarray(a) for a in self.sharded(*concat_in,
                                                        *concat_zeros)]
        return [
            {name: out_arrs[i].reshape(n, *self.zero_shapes[i][0])[c]
             for i, name in enumerate(self.out_names)}
            for c in range(n)
        ]


_EXEC = None


def _get_exec():
    global _EXEC
    if _EXEC is None:
        nc = _build()
        _EXEC = _CachedSpmdExec(nc, N_CORES)
    return _EXEC


def _shard_inputs(x, y):
    x = np.ascontiguousarray(np.asarray(x, dtype=np.float32))
    y = np.asarray(y).astype(np.int32)
    R = B_FULL // N_CORES
    nb = R // 128
    iota2 = np.ascontiguousarray(
        np.broadcast_to(np.arange(C_FULL, dtype=np.int16), (128, C_FULL)))
    ident = np.eye(128, dtype=np.float32)
    in_maps = []
    for k in range(N_CORES):
        xs = x[k * R:(k + 1) * R]
        ys = np.ascontiguousarray(y[k * R:(k + 1) * R].reshape(nb, 128).T)
        in_maps.append({"x": xs, "yt": ys, "iota2": iota2, "ident": ident})
    return in_maps


def kernel(x, y):
    """Full inputs in, full output out (distributes over 8 cores inside)."""
    x = np.asarray(x)
    y = np.asarray(y)
    assert x.shape == (B_FULL, C_FULL) and y.shape == (B_FULL,)
    ex = _get_exec()
    res = ex(_shard_inputs(x, y))
    out = np.asarray(res[0]["out"]).reshape(-1)[0]
    return np.float32(out)

